# revision 1
# baseline (speedup 1.0000x reference)
"""Trainium2 Bass kernel for the 14-term hydrogen-orbital basis evaluation.

Computes out[i,j] = sum_k coeffs[k] * R_{n_k l_k}(r) * Y_{l_k m_k}(theta, phi)
for position (2048, 4096, 3) = (r, theta, phi), distributed pure data-parallel
across 8 NeuronCores (256 rows of OutN each; coeffs folded host-side).

Math: the 14-term sum is refactored host-side (coeffs are runtime inputs, but
only 14 scalars) into

  out = E2*(a1*E2 + a2 + r*A) + E3*(a4 + r*B + r^2*C)
  A = a3 + w11*u + s*ro1*sin(phi+d1)
  B = a5 + w12*u + s*ro2*sin(phi+d2)
  C = c0q + c1*u + c2*u^2 + s*(ro3*sin(phi+d3) + ro4*u*sin(phi+d4))
      + (1-u^2)*ro5*sin(2*phi+d5)

with E2 = exp(-r/2), E3 = exp(-r/3), u = cos(theta), s = sqrt(max(1-u^2,0)).
All sin/cos pairs are phase-folded into single ACT Sin lookups (phases kept
inside the table's [-pi, pi] domain), including the merged
ro3*sin(phi+d3) + ro4*sin(phi+d4) pair.  u = cos(theta) = 1 - 2*hh with
hh = sin^2(theta/2) is never materialized: affine consumers read hh directly
(power-of-two scale, bit-identical), and u^2 is one ACT op,
Square(-2*hh + 1).  This exactly reproduces the fp32 reference's rounding of
cos(theta) near theta -> 0, where s = sqrt(1-u^2) amplifies any u mismatch
catastrophically.  E2/E3 come from the ACT Exp table (~1e-5 relative table
error, the dominant error term; a higher-accuracy degree-5 exp(-r/6)
polynomial variant is available via cfg["e6poly"] at ~+65%% runtime).
Elementwise work is spread across the Vector, GPSIMD and Scalar(ACT) engines
at a measured three-way load equilibrium, tuned with the TimelineSim cost
model (~175 us per core predicted; ideal-balance floor ~118 us).

Environment notes: this container's walrus rejects the CUSTOM_DVE_ANT
extension and Pool scalar_tensor_tensor, so only stock engine ops are used.
Hardware CTRL/DMA instructions accept a single sync wait, so a BIR post-pass
splits Tile's multi-wait tail drain onto NoOps (_split_excess_waits).
"""

import math

import numpy as np

COLS = 8192  # per-core elements per partition row: 256*4096/128
P = 128
N_CORES = 8
F_BLK = 1024  # columns per processing block
WK_BUFS = 22  # shared rotating slots for per-block tensors

# exp(-r/6) on [0,1], degree-5 (chebyshev-fit, power basis)
_E6C = [
    1.0000000001659477,
    -0.16666666530963805,
    0.013888876279549364,
    -7.715543225269446e-04,
    3.205458211837887e-05,
    -9.862506313437053e-07,
]

_BUILD_CACHE = {}
LAST_RESULTS = None


# --------------------------------------------------------------------------
# host-side math: fold the 14 coeffs into the factored-formula weights
# --------------------------------------------------------------------------
def _derive_weights(coeffs):
    c = np.asarray(coeffs, dtype=np.float64)

    def rad_norm(n, l):
        return math.sqrt(
            (2.0 / n) ** 3
            * math.factorial(n - l - 1)
            / (2.0 * n * math.factorial(n + l))
        )

    n10 = rad_norm(1, 0)
    n20 = rad_norm(2, 0)
    n21 = rad_norm(2, 1)
    n30 = rad_norm(3, 0)
    n31 = rad_norm(3, 1)
    n32 = rad_norm(3, 2)

    def sph_norm(l, m):
        am = abs(m)
        return math.sqrt(
            (2 * l + 1)
            / (4.0 * math.pi)
            * math.factorial(l - am)
            / math.factorial(l + am)
        )

    k00 = sph_norm(0, 0)
    k10 = sph_norm(1, 0)
    k11 = sph_norm(1, 1)
    k20 = sph_norm(2, 0)
    k21 = sph_norm(2, 1)
    k22 = sph_norm(2, 2)
    s2 = math.sqrt(2.0)

    g32 = n32 * 4.0 / 9.0  # R32 = g32 * r^2 * E3
    G = c[11] * k20 * g32

    W = {}
    W["a1"] = k00 * c[0] * n10
    W["a2"] = k00 * c[1] * n20 * 2.0
    W["a3"] = -k00 * c[1] * n20
    W["a4"] = k00 * c[5] * n30 * 3.0
    W["a5"] = -k00 * c[5] * n30 * 2.0
    W["c0q"] = k00 * c[5] * n30 * 2.0 / 9.0 - G / 2.0
    W["w11"] = k10 * c[3] * n21
    W["w12"] = k10 * c[7] * n31 * 8.0 / 3.0
    W["c1"] = -k10 * c[7] * n31 * 4.0 / 9.0
    W["c2"] = 1.5 * G

    x_rE2 = -s2 * k11 * n21 * c[2]
    y_rE2 = -s2 * k11 * n21 * c[4]
    x_rE3 = -s2 * k11 * n31 * 8.0 / 3.0 * c[6]
    y_rE3 = -s2 * k11 * n31 * 8.0 / 3.0 * c[8]
    x_r2E3_c = s2 * k11 * n31 * 4.0 / 9.0 * c[6]
    y_r2E3_c = s2 * k11 * n31 * 4.0 / 9.0 * c[8]
    x_r2E3_u = -3.0 * s2 * k21 * g32 * c[10]
    y_r2E3_u = -3.0 * s2 * k21 * g32 * c[12]
    z1 = 3.0 * s2 * k22 * g32 * c[9]
    z2 = 3.0 * s2 * k22 * g32 * c[13]

    def fold(x, y, span):
        # x*sin(t) + y*cos(t) = rho*sin(t + d); keep args t+d within the ACT
        # Sin table's [-pi, pi] domain for t in [0, span] by flipping by pi.
        rho = math.hypot(x, y)
        d = math.atan2(y, x)
        if d + span > math.pi:
            d -= math.pi
            rho = -rho
        return rho, d

    W["ro1"], W["d1"] = fold(x_rE2, y_rE2, 1.0)
    W["ro2"], W["d2"] = fold(x_rE3, y_rE3, 1.0)
    W["ro3"], W["d3"] = fold(x_r2E3_c, y_r2E3_c, 1.0)
    W["ro4"], W["d4"] = fold(x_r2E3_u, y_r2E3_u, 1.0)
    W["ro5"], W["d5"] = fold(z1, z2, 2.0)
    return {k: float(v) for k, v in W.items()}


# --------------------------------------------------------------------------
# BIR post-pass: hardware allows a single sync-wait per instruction; Tile's
# tail drain can carry several.  Split extras onto preceding same-engine NoOps.
# --------------------------------------------------------------------------
def _split_excess_waits(nc, max_waits=1):
    import concourse.mybir as mybir

    for bb in nc.m.functions[0].blocks:
        insts = bb.instructions
        i = 0
        while i < len(insts):
            inst = insts[i]
            si = getattr(inst, "sync_info", None)
            waits = list(si.on_wait) if si is not None and si.on_wait else []
            if len(waits) > max_waits:
                keep = waits[:max_waits]
                extra = waits[max_waits:]
                chunks = [
                    extra[j : j + max_waits] for j in range(0, len(extra), max_waits)
                ]
                new_insts = []
                for ci, ch in enumerate(chunks):
                    nop = mybir.InstNoOp(
                        name=f"{inst.name}-wsplit-{ci}",
                        engine=inst.engine,
                        ins=[],
                        outs=[],
                        sync_info=mybir.SyncInfo(on_wait=ch, on_update=[]),
                    )
                    nc.register_instruction(nop, overwrite=True)
                    new_insts.append(nop)
                inst.sync_info = mybir.SyncInfo(
                    on_wait=keep,
                    on_update=list(si.on_update) if si.on_update else [],
                )
                for k, ni in enumerate(new_insts):
                    insts.insert(i + k, ni)
                i += len(new_insts)
            i += 1


# --------------------------------------------------------------------------
# kernel builder (stock engine ops only - this container's walrus rejects
# the CUSTOM_DVE_ANT extension, and Pool rejects scalar_tensor_tensor)
# --------------------------------------------------------------------------
def _build_nc(W, cfg=None):
    import concourse.bass as bass
    import concourse.mybir as mybir
    from concourse import tile

    AF = mybir.ActivationFunctionType
    MULT = mybir.AluOpType.mult
    ADD = mybir.AluOpType.add
    f32 = mybir.dt.float32

    cfg = dict(cfg or {})
    f_blk = cfg.get("f_blk", F_BLK)
    sbk = cfg.get("sbk", 1)  # super-block: ACT table phases batched over sbk blocks
    wk_bufs = cfg.get("wk_bufs", WK_BUFS)
    # tt-class op placement: "v" = vector, "p" = gpsimd/pool
    tt_eng = {
        "hh": "p", "A": "p", "B": "p", "K34": "v", "Z5": "p", "Cu": "v",
        "C": "v", "RA": "p", "RC": "p", "BRC": "v", "t3": "v", "t4": "v",
        "F2": "v", "out": "v", "J34": "v",
    }
    tt_eng.update(cfg.get("tt", {}))
    # affine sites: "a" = ACT Identity, "v" = vector ts, "p" = pool ts
    aff_eng = {"u": "p", "w5": "a", "Cq": "a", "t2": "v", "Ap": "a", "Bp": "a"}
    aff_eng.update(cfg.get("aff", {}))
    # stt sites: "f" = fused scalar_tensor_tensor on vector; "x.y" = split into
    # an affine pre-op on engine x (a/v/p) + tensor_tensor on engine y (v/p)
    stt_eng = {
        "K1": "f", "K2": "f", "uS4": "f", "J34": "f", "Cq2": "f",
        "t": "f", "F3": "f",
    }
    stt_eng.update(cfg.get("stt", {}))

    # activation bias lookup is by exact fp32 value - round everything once
    W = {k: float(np.float32(v)) for k, v in W.items()}

    nc = bass.Bass()
    pos = nc.dram_tensor("pos", [P, 3 * COLS], f32, kind="ExternalInput")
    out_d = nc.dram_tensor("out", [P, COLS], f32, kind="ExternalOutput")

    def reg_const(val):
        val = float(np.float32(val))
        key = (f32, val)
        if key not in nc.const_aps.aps:
            t = nc.alloc_sbuf_tensor(f"cst{len(nc.const_aps.aps)}", [P, 1], f32)
            nc.gpsimd.memset(t.ap(), val)
            nc.const_aps.aps[key] = t.ap()
        return val

    # merge ro3*sin(phi+d3) + ro4*sin(phi+d4) into one sin (the u-dependent
    # part of the pair is handled separately via hh = sin^2(theta/2))
    xc = W["ro3"] * math.cos(W["d3"]) + W["ro4"] * math.cos(W["d4"])
    yc = W["ro3"] * math.sin(W["d3"]) + W["ro4"] * math.sin(W["d4"])
    ro34 = math.hypot(xc, yc)
    d34 = math.atan2(yc, xc)
    if d34 + 1.0 > math.pi:
        d34 -= math.pi
        ro34 = -ro34
    W["ro34"] = float(np.float32(ro34))
    W["d34"] = float(np.float32(d34))
    # hh-affine equivalents of the u-affine sites (u = 1 - 2*hh exactly)
    W["apb"] = float(np.float32(W["w11"] + W["a3"]))
    W["bpb"] = float(np.float32(W["w12"] + W["a5"]))
    W["cqb"] = float(np.float32(W["c1"] + W["c0q"]))

    e6 = [float(np.float32(x)) for x in _E6C]
    for v in (W["d1"], W["d2"], W["d34"], W["d4"], W["d5"], 1.0,
              W["ro5"], W["a2"], W["a3"], W["a5"],
              W["apb"], W["bpb"], W["cqb"],
              e6[0], e6[1], e6[3], e6[4]):
        reg_const(v)
    nc.all_engine_barrier()

    def reg_const2(v):
        v = float(np.float32(v))
        assert (f32, v) in nc.const_aps.aps, f"bias {v} not preregistered"
        return v

    NB = COLS // f_blk

    with tile.TileContext(nc) as tc:
        with (
            tc.tile_pool(name="io", bufs=2) as io,
            tc.tile_pool(name="wk", bufs=wk_bufs) as wk,
        ):
            NSB = NB // sbk
            repeat = cfg.get("repeat", 1)
            for sb in range(NSB * repeat):
                sb = sb % NSB
                blocks = []
                for j in range(sbk):
                    b = sb * sbk + j
                    cf = b * f_blk
                    slab = io.tile([P, 3 * f_blk], f32, tag="slab", name=f"slab{b}")
                    nc.sync.dma_start(slab[:], pos[:, 3 * cf : 3 * (cf + f_blk)])
                    v3 = slab.rearrange("p (n c) -> p n c", c=3)
                    blocks.append(
                        dict(cf=cf, r=v3[:, :, 0], th=v3[:, :, 1], ph=v3[:, :, 2], t={})
                    )

                def T(blk, tagname):
                    tl = wk.tile([P, f_blk], f32, tag="wk", name=tagname)
                    blk["t"][tagname] = tl
                    return tl

                def TT(site, out, a, b_, op):
                    eng = nc.vector if tt_eng[site] == "v" else nc.gpsimd
                    if op == "mul":
                        eng.tensor_mul(out, a, b_)
                    else:
                        eng.tensor_add(out, a, b_)

                def AFF(site, out, inp, scale, bias):
                    e = aff_eng[site]
                    if e == "a":
                        nc.scalar.activation(
                            out, inp, AF.Identity, bias=reg_const2(bias), scale=scale
                        )
                    else:
                        eng = nc.vector if e == "v" else nc.gpsimd
                        eng.tensor_scalar(out, inp, scale, bias, MULT, ADD)

                def STT(site, blk, out, in0, scalar, in1, op1):
                    mode = stt_eng[site]
                    if mode == "f":
                        nc.vector.scalar_tensor_tensor(out, in0, scalar, in1, MULT, op1)
                        return
                    pre_e, tt_e = mode.split(".")
                    pre = T(blk, site + "_pre")
                    if pre_e == "a":
                        nc.scalar.activation(
                            pre[:], in0, AF.Identity, bias=0.0, scale=scalar
                        )
                    else:
                        eng = nc.vector if pre_e == "v" else nc.gpsimd
                        eng.tensor_scalar(pre[:], in0, scalar, 0.0, MULT, ADD)
                    eng = nc.vector if tt_e == "v" else nc.gpsimd
                    if op1 is MULT:
                        eng.tensor_mul(out, pre[:], in1)
                    else:
                        eng.tensor_add(out, pre[:], in1)

                # ---- phase 1: radial exponentials ----
                if not cfg.get("e6poly"):
                    # ACT exp table (fast, ~1e-5 relative table error)
                    for blk in blocks:
                        E2 = T(blk, "E2")
                        nc.scalar.activation(E2[:], blk["r"], AF.Exp, scale=-0.5)
                        E3 = T(blk, "E3")
                        nc.scalar.activation(
                            E3[:], blk["r"], AF.Exp, scale=float(np.float32(-1.0 / 3.0))
                        )
                else:
                    # degree-5 polynomial for E6=exp(-r/6); E2=E6^3, E3=E6^2.
                    # ~3e-7 relative, and drops the exp table set entirely.
                    for blk in blocks:
                        r_ = blk["r"]
                        Qa = T(blk, "Qa")
                        nc.scalar.activation(
                            Qa[:], r_, AF.Identity, bias=e6[4], scale=e6[5]
                        )
                        Qb = T(blk, "Qb")
                        nc.gpsimd.tensor_mul(Qb[:], Qa[:], r_)
                        Qc = T(blk, "Qc")
                        nc.scalar.activation(Qc[:], Qb[:], AF.Identity, bias=e6[3], scale=1.0)
                        Pa = T(blk, "Pa")
                        nc.scalar.activation(
                            Pa[:], r_, AF.Identity, bias=e6[1], scale=e6[2]
                        )
                        Pb = T(blk, "Pb")
                        nc.gpsimd.tensor_mul(Pb[:], Pa[:], r_)
                        Pc = T(blk, "Pc")
                        nc.scalar.activation(Pc[:], Pb[:], AF.Identity, bias=e6[0], scale=1.0)
                        r2 = T(blk, "r2")
                        nc.vector.tensor_mul(r2[:], r_, r_)
                        r3 = T(blk, "r3")
                        nc.gpsimd.tensor_mul(r3[:], r2[:], r_)
                        Qr = T(blk, "Qr")
                        nc.vector.tensor_mul(Qr[:], Qc[:], r3[:])
                        E6 = T(blk, "E6")
                        nc.vector.tensor_add(E6[:], Pc[:], Qr[:])
                        E3 = T(blk, "E3")
                        nc.vector.tensor_mul(E3[:], E6[:], E6[:])
                        E2 = T(blk, "E2")
                        nc.gpsimd.tensor_mul(E2[:], E3[:], E6[:])

                # ---- phase 2: trig table set ----
                for blk in blocks:
                    h = T(blk, "h")
                    nc.scalar.activation(h[:], blk["th"], AF.Sin, scale=0.5)
                    for i, d in ((1, "d1"), (2, "d2"), (3, "d34"), (4, "d4")):
                        S = T(blk, f"S{i}")
                        nc.scalar.activation(S[:], blk["ph"], AF.Sin, bias=W[d])
                    S5 = T(blk, "S5")
                    nc.scalar.activation(S5[:], blk["ph"], AF.Sin, bias=W["d5"], scale=2.0)

                # ---- u = cos(theta) = 1 - 2*sin^2(theta/2) ----
                for blk in blocks:
                    hh = T(blk, "hh")
                    if tt_eng["hh"] == "a":
                        nc.scalar.activation(hh[:], blk["t"]["h"][:], AF.Square)
                    else:
                        TT("hh", hh[:], blk["t"]["h"][:], blk["t"]["h"][:], "mul")

                # ---- phase 3: sqrt table set ----
                # u = 1 - 2*hh is never materialized: the -2x+1 affine (exact,
                # power-of-two scale) rides inside Square's input transform.
                for blk in blocks:
                    U2 = T(blk, "U2")
                    nc.scalar.activation(
                        U2[:], blk["t"]["hh"][:], AF.Square, bias=1.0, scale=-2.0
                    )
                    s = T(blk, "s")
                    nc.scalar.activation(s[:], U2[:], AF.Sqrt, bias=1.0, scale=-1.0)

                # ---- per-block DVE/pool chain ----
                for blk in blocks:
                    g = blk["t"]
                    r_, hh, s, U2 = blk["r"], g["hh"], g["s"], g["U2"]
                    E2, E3, S5 = g["E2"], g["E3"], g["S5"]
                    K1 = T(blk, "K1")
                    STT("K1", blk, K1[:], g["S1"][:], W["ro1"], s[:], MULT)
                    Ap = T(blk, "Ap")
                    AFF("Ap", Ap[:], hh[:], -2.0 * W["w11"], W["apb"])
                    A = T(blk, "A")
                    TT("A", A[:], Ap[:], K1[:], "add")
                    K2 = T(blk, "K2")
                    STT("K2", blk, K2[:], g["S2"][:], W["ro2"], s[:], MULT)
                    Bp = T(blk, "Bp")
                    AFF("Bp", Bp[:], hh[:], -2.0 * W["w12"], W["bpb"])
                    B = T(blk, "B")
                    TT("B", B[:], Bp[:], K2[:], "add")
                    uS4 = T(blk, "uS4")
                    STT("uS4", blk, uS4[:], g["S4"][:], -2.0 * W["ro4"], hh[:], MULT)
                    J34 = T(blk, "J34")
                    STT("J34", blk, J34[:], g["S3"][:], W["ro34"], uS4[:], ADD)
                    K34 = T(blk, "K34")
                    TT("K34", K34[:], s[:], J34[:], "mul")
                    w5 = T(blk, "w5")
                    AFF("w5", w5[:], U2[:], -W["ro5"], W["ro5"])
                    Z5 = T(blk, "Z5")
                    TT("Z5", Z5[:], w5[:], S5[:], "mul")
                    Cq = T(blk, "Cq")
                    AFF("Cq", Cq[:], hh[:], -2.0 * W["c1"], W["cqb"])
                    Cq2 = T(blk, "Cq2")
                    STT("Cq2", blk, Cq2[:], U2[:], W["c2"], Cq[:], ADD)
                    Cu = T(blk, "Cu")
                    TT("Cu", Cu[:], K34[:], Cq2[:], "add")
                    C = T(blk, "C")
                    TT("C", C[:], Cu[:], Z5[:], "add")
                    RA = T(blk, "RA")
                    TT("RA", RA[:], r_, A[:], "mul")
                    t = T(blk, "t")
                    STT("t", blk, t[:], E2[:], W["a1"], RA[:], ADD)
                    t2 = T(blk, "t2")
                    AFF("t2", t2[:], t[:], 1.0, W["a2"])
                    F2 = T(blk, "F2")
                    TT("F2", F2[:], t2[:], E2[:], "mul")
                    RC = T(blk, "RC")
                    TT("RC", RC[:], r_, C[:], "mul")
                    BRC = T(blk, "BRC")
                    TT("BRC", BRC[:], B[:], RC[:], "add")
                    t3 = T(blk, "t3")
                    TT("t3", t3[:], r_, BRC[:], "mul")
                    t4 = T(blk, "t4")
                    TT("t4", t4[:], E3[:], t3[:], "mul")
                    F3 = T(blk, "F3")
                    STT("F3", blk, F3[:], E3[:], W["a4"], t4[:], ADD)
                    ot = io.tile([P, f_blk], f32, tag="ot", name=f"ot{blk['cf']}")
                    TT("out", ot[:], F2[:], F3[:], "add")
                    nc.sync.dma_start(out_d[:, blk["cf"] : blk["cf"] + f_blk], ot[:])

    _split_excess_waits(nc, 1)
    return nc


# --------------------------------------------------------------------------
# public entry point
# --------------------------------------------------------------------------
def kernel(position, coeffs):
    global LAST_RESULTS
    from concourse.bass_utils import run_bass_kernel_spmd

    position = np.ascontiguousarray(np.asarray(position, dtype=np.float32))
    coeffs = np.asarray(coeffs, dtype=np.float32)
    OutN, n, _ = position.shape
    rows = OutN // N_CORES

    key = coeffs.tobytes()
    if key not in _BUILD_CACHE:
        _BUILD_CACHE[key] = _build_nc(_derive_weights(coeffs))
    nc = _BUILD_CACHE[key]

    in_maps = []
    for c in range(N_CORES):
        shard = position[c * rows : (c + 1) * rows].reshape(P, COLS * 3)
        in_maps.append({"pos": np.ascontiguousarray(shard)})

    res = None
    last_exc = None
    for attempt in range(3):
        try:
            res = run_bass_kernel_spmd(nc, in_maps, core_ids=list(range(N_CORES)))
            break
        except Exception as exc:  # wedged-device resilience: retry fresh
            last_exc = exc
            import time as _time

            _time.sleep(10)
    if res is None:
        raise last_exc
    LAST_RESULTS = res
    out = np.empty((OutN, n), dtype=np.float32)
    for c in range(N_CORES):
        out[c * rows : (c + 1) * rows] = res.results[c]["out"].reshape(rows, n)
    return out



# revision 22
# speedup vs baseline: 1.6052x; 1.6052x over previous
"""Trainium2 Bass kernel for the 14-term hydrogen-orbital basis evaluation.

Computes out[i,j] = sum_k coeffs[k] * R_{n_k l_k}(r) * Y_{l_k m_k}(theta, phi)
for position (2048, 4096, 3) = (r, theta, phi), distributed pure data-parallel
across 8 NeuronCores (256 rows of OutN each; coeffs folded host-side).

Math (coeffs are runtime inputs, but only 14 scalars -> folded host-side):

  out = E2*(a1*E2 + a2 + r*A) + E3*(a4 + r*(B + r*C))
  A = a3 + w11*u + ro1*s*S1
  B = a5 + w12*u + ro2*s*S2
  C = (c2*u + c1)*u + c0q + s*(ro34*S34 + ro4*(u-1)*S4 + ro5*s*S5)

with E2 = exp(-r/2), E3 = exp(-r/3), u = cos(theta) = Sin(theta + pi/2),
s = sin(theta) (NOT sqrt(1-u^2): theta in [0,1) so s = sin(theta) exactly,
avoiding the catastrophic cancellation near theta->0 entirely), and
Sk = sin(phi + dk) phase-folded amplitude/phase pairs (kept inside the ACT
Sin table's [-pi, pi] domain).

The whole pipeline runs in fp16: the DVE gets 2x throughput for packed
2-byte tensor_tensor ops and 4x for tensor_scalar affines, which more than
doubles the op budget vs the fp32 formulation, while fp16 rounding after
every op keeps |err|/absmax ~3e-3 (tolerance 2e-2).  scalar_tensor_tensor
gets NO 16-bit speedup, so every former STT site is split into a
tensor_scalar (0.26 ns/elem) + tensor_tensor (0.52 ns/elem) pair.  Inputs
are de-interleaved host-side into planar fp16 r/theta/phi (halves DMA), and
the output returns as fp16 (upcast host-side).

Elementwise work is spread across Vector, GPSIMD and Scalar(ACT) at a
TimelineSim-tuned balance; per-site engine maps are in cfg.

Environment notes: this container's walrus rejects the CUSTOM_DVE_ANT
extension and Pool scalar_tensor_tensor, so only stock engine ops are used.
Hardware CTRL/DMA instructions accept a single sync wait, so a BIR post-pass
splits Tile's multi-wait tail drain onto NoOps (_split_excess_waits).
"""

import math

import numpy as np

COLS = 8192  # per-core elements per partition row: 256*4096/128
P = 128
N_CORES = 8
F_BLK = 2048  # columns per processing block
WK_BUFS = 28  # shared rotating slots for per-block tensors

_BUILD_CACHE = {}
LAST_RESULTS = None

# engine-placement / pipelining config (tuned against TimelineSim)
DEFAULT_CFG = {
    "f_blk": 1024,
    "io_bufs": 6,
    "lag": 1,
    "def_bufs": 3,
    "odma": "s",
    "ts": {"J34s": "a", "Cc": "a"},
    "tt": {"sJz": "v", "J": "v", "F2": "p", "i2": "p", "Jz": "v", "B": "p"},
    "bufs": {
        "S4": 6, "S5": 6, "s": 6, "u": 5, "S1": 4, "S2": 4, "S34": 4,
        "E2": 4, "E3": 4, "um": 4, "J34s": 4, "Z5p": 4, "Au": 3, "Bu": 3,
        "sS1": 4, "sS2": 4, "m1": 4, "t": 4,
    },
}


# --------------------------------------------------------------------------
# host-side math: fold the 14 coeffs into the factored-formula weights
# --------------------------------------------------------------------------
def _derive_weights(coeffs):
    c = np.asarray(coeffs, dtype=np.float64)

    def rad_norm(n, l):
        return math.sqrt(
            (2.0 / n) ** 3
            * math.factorial(n - l - 1)
            / (2.0 * n * math.factorial(n + l))
        )

    n10 = rad_norm(1, 0)
    n20 = rad_norm(2, 0)
    n21 = rad_norm(2, 1)
    n30 = rad_norm(3, 0)
    n31 = rad_norm(3, 1)
    n32 = rad_norm(3, 2)

    def sph_norm(l, m):
        am = abs(m)
        return math.sqrt(
            (2 * l + 1)
            / (4.0 * math.pi)
            * math.factorial(l - am)
            / math.factorial(l + am)
        )

    k00 = sph_norm(0, 0)
    k10 = sph_norm(1, 0)
    k11 = sph_norm(1, 1)
    k20 = sph_norm(2, 0)
    k21 = sph_norm(2, 1)
    k22 = sph_norm(2, 2)
    s2 = math.sqrt(2.0)

    g32 = n32 * 4.0 / 9.0  # R32 = g32 * r^2 * E3
    G = c[11] * k20 * g32

    W = {}
    W["a1"] = k00 * c[0] * n10
    W["a2"] = k00 * c[1] * n20 * 2.0
    W["a3"] = -k00 * c[1] * n20
    W["a4"] = k00 * c[5] * n30 * 3.0
    W["a5"] = -k00 * c[5] * n30 * 2.0
    W["c0q"] = k00 * c[5] * n30 * 2.0 / 9.0 - G / 2.0
    W["w11"] = k10 * c[3] * n21
    W["w12"] = k10 * c[7] * n31 * 8.0 / 3.0
    W["c1"] = -k10 * c[7] * n31 * 4.0 / 9.0
    W["c2"] = 1.5 * G

    x_rE2 = -s2 * k11 * n21 * c[2]
    y_rE2 = -s2 * k11 * n21 * c[4]
    x_rE3 = -s2 * k11 * n31 * 8.0 / 3.0 * c[6]
    y_rE3 = -s2 * k11 * n31 * 8.0 / 3.0 * c[8]
    x_r2E3_c = s2 * k11 * n31 * 4.0 / 9.0 * c[6]
    y_r2E3_c = s2 * k11 * n31 * 4.0 / 9.0 * c[8]
    x_r2E3_u = -3.0 * s2 * k21 * g32 * c[10]
    y_r2E3_u = -3.0 * s2 * k21 * g32 * c[12]
    z1 = 3.0 * s2 * k22 * g32 * c[9]
    z2 = 3.0 * s2 * k22 * g32 * c[13]

    def fold(x, y, span):
        # x*sin(t) + y*cos(t) = rho*sin(t + d); keep args t+d within the ACT
        # Sin table's [-pi, pi] domain for t in [0, span] by flipping by pi.
        rho = math.hypot(x, y)
        d = math.atan2(y, x)
        if d + span > math.pi:
            d -= math.pi
            rho = -rho
        return rho, d

    W["ro1"], W["d1"] = fold(x_rE2, y_rE2, 1.0)
    W["ro2"], W["d2"] = fold(x_rE3, y_rE3, 1.0)
    W["ro3"], W["d3"] = fold(x_r2E3_c, y_r2E3_c, 1.0)
    W["ro4"], W["d4"] = fold(x_r2E3_u, y_r2E3_u, 1.0)
    W["ro5"], W["d5"] = fold(z1, z2, 2.0)
    return {k: float(v) for k, v in W.items()}


# --------------------------------------------------------------------------
# BIR post-pass: hardware allows a single sync-wait per instruction; Tile's
# tail drain can carry several.  Split extras onto preceding same-engine NoOps.
# --------------------------------------------------------------------------
def _split_excess_waits(nc, max_waits=1):
    import concourse.mybir as mybir

    for bb in nc.m.functions[0].blocks:
        insts = bb.instructions
        i = 0
        while i < len(insts):
            inst = insts[i]
            si = getattr(inst, "sync_info", None)
            waits = list(si.on_wait) if si is not None and si.on_wait else []
            if len(waits) > max_waits:
                keep = waits[:max_waits]
                extra = waits[max_waits:]
                chunks = [
                    extra[j : j + max_waits] for j in range(0, len(extra), max_waits)
                ]
                new_insts = []
                for ci, ch in enumerate(chunks):
                    nop = mybir.InstNoOp(
                        name=f"{inst.name}-wsplit-{ci}",
                        engine=inst.engine,
                        ins=[],
                        outs=[],
                        sync_info=mybir.SyncInfo(on_wait=ch, on_update=[]),
                    )
                    nc.register_instruction(nop, overwrite=True)
                    new_insts.append(nop)
                inst.sync_info = mybir.SyncInfo(
                    on_wait=keep,
                    on_update=list(si.on_update) if si.on_update else [],
                )
                for k, ni in enumerate(new_insts):
                    insts.insert(i + k, ni)
                i += len(new_insts)
            i += 1


# --------------------------------------------------------------------------
# kernel builder
# --------------------------------------------------------------------------
def _build_nc(W, cfg=None):
    import concourse.bass as bass
    import concourse.mybir as mybir
    from concourse import tile

    AF = mybir.ActivationFunctionType
    MULT = mybir.AluOpType.mult
    ADD = mybir.AluOpType.add
    f16 = mybir.dt.float16
    f32 = mybir.dt.float32

    cfg = dict(cfg or {})
    f_blk = cfg.get("f_blk", F_BLK)
    wk_bufs = cfg.get("wk_bufs", WK_BUFS)
    io_bufs = cfg.get("io_bufs", 2)
    # tensor_tensor site placement: "v" = vector(DVE), "p" = gpsimd/pool.
    # the whole J-chain lives on Pool: self-contained, fed by ACT tables and
    # three early DVE tensor_scalars, with a single handoff out (sJz)
    tt_eng = {
        "sS1": "v", "sS2": "v", "A": "v", "B": "v", "uS4": "p", "J": "p",
        "z1": "p", "Jz": "p", "sJz": "p", "m2": "v", "C": "v", "rA": "v",
        "i2": "v", "F2": "v", "H1": "v", "H2": "v", "H3": "v", "F3": "v",
        "out": "v",
    }
    tt_eng.update(cfg.get("tt", {}))
    # tensor_scalar (affine) site placement: "v" = DVE, "a" = ACT Identity,
    # "p" = pool tensor_scalar
    ts_eng = {
        "t": "v", "Au": "v", "Bu": "v", "K1": "v", "K2": "v", "um": "v",
        "J34s": "v", "Z5p": "v", "m1": "v", "Cc": "v", "i3": "v",
    }
    ts_eng.update(cfg.get("ts", {}))
    # per-tensor ring depth overrides
    tile_bufs = dict(cfg.get("bufs", {}))

    # activation bias lookup is by exact fp32 value - round everything once
    W = {k: float(np.float32(v)) for k, v in W.items()}

    # merge ro3*sin(phi+d3) + ro4*sin(phi+d4) into one sin (the u-dependent
    # part of the pair is handled via ro4*(u-1)*S4)
    xc = W["ro3"] * math.cos(W["d3"]) + W["ro4"] * math.cos(W["d4"])
    yc = W["ro3"] * math.sin(W["d3"]) + W["ro4"] * math.sin(W["d4"])
    ro34 = math.hypot(xc, yc)
    d34 = math.atan2(yc, xc)
    if d34 + 1.0 > math.pi:
        d34 -= math.pi
        ro34 = -ro34
    W["ro34"] = float(np.float32(ro34))
    W["d34"] = float(np.float32(d34))
    HPI = float(np.float32(math.pi / 2.0))

    pack_in = cfg.get("pack_in", False)
    pe_out = cfg.get("pe_out", False)

    nc = bass.Bass()
    if pack_in:
        pos_d = nc.dram_tensor("pos", [P, 3 * COLS], f16, kind="ExternalInput")
        pos3 = pos_d.ap().rearrange("p (c n) -> p c n", c=3)
    else:
        r_d = nc.dram_tensor("r", [P, COLS], f16, kind="ExternalInput")
        th_d = nc.dram_tensor("th", [P, COLS], f16, kind="ExternalInput")
        ph_d = nc.dram_tensor("ph", [P, COLS], f16, kind="ExternalInput")
    out_d = nc.dram_tensor("out", [P, COLS], f32 if pe_out else f16,
                           kind="ExternalOutput")
    if pe_out:
        eye_d = nc.dram_tensor("eye", [P, P], f16, kind="ExternalInput")
        eye_s = nc.alloc_sbuf_tensor("eye_s", [P, P], f16)
        nc.sync.dma_start(eye_s.ap(), eye_d.ap())

    def reg_const(val):
        val = float(np.float32(val))
        key = (f32, val)
        if key not in nc.const_aps.aps:
            t = nc.alloc_sbuf_tensor(f"cst{len(nc.const_aps.aps)}", [P, 1], f32)
            nc.gpsimd.memset(t.ap(), val)
            nc.const_aps.aps[key] = t.ap()
        return val

    # table biases + any ACT-assigned affine biases
    for v in (W["d1"], W["d2"], W["d34"], W["d4"], W["d5"], HPI,
              W["a2"], W["a3"], W["a5"], W["c1"], W["c0q"], W["a4"]):
        reg_const(v)
    nc.all_engine_barrier()

    def reg_const2(v):
        v = float(np.float32(v))
        assert (f32, v) in nc.const_aps.aps, f"bias {v} not preregistered"
        return v

    NB = COLS // f_blk
    lag = cfg.get("lag", 1)  # software-pipeline distance between head and tail
    def_bufs = cfg.get("def_bufs", 1)  # ring depth for ordinary tiles

    # tensors read by the lagged tail stage need extra ring slots so the head
    # of block b+lag doesn't overwrite what block b's tail still reads
    # (tags named by their root tensor; in-place chains extend live ranges:
    # sS1 carries A/F2, sS2 carries B, m1 carries Cc/C/H*, um carries sJz)
    crossing = {"sS1", "sS2", "m1", "um", "t", "E2", "E3"}
    crossing.update(cfg.get("crossing", ()))

    with tile.TileContext(nc) as tc:
        with (
            tc.tile_pool(name="io", bufs=2) as io,
            tc.tile_pool(name="wk", bufs=wk_bufs) as wk,
        ):
            G = [dict() for _ in range(NB)]

            def T(b, name):
                nb = tile_bufs.get(
                    name, (def_bufs + lag) if name in crossing else def_bufs
                )
                tl = wk.tile([P, f_blk], f16, tag=name, bufs=nb, name=name)
                G[b][name] = tl
                return tl

            def ACTI(b, name, src, fn, scale=1.0, bias=0.0):
                o = T(b, name)
                if bias:
                    nc.scalar.activation(
                        o[:], G[b][src][:], fn, bias=reg_const2(bias), scale=scale
                    )
                else:
                    nc.scalar.activation(o[:], G[b][src][:], fn, scale=scale)

            def TT(b, name, a, b_, op="m", into=None):
                # into=X reuses X's tile in place (X must be dead after this)
                o = G[b][into] if into else T(b, name)
                if into:
                    G[b][name] = o
                eng = nc.vector if tt_eng[name] == "v" else nc.gpsimd
                if op == "m":
                    eng.tensor_mul(o[:], G[b][a][:], G[b][b_][:])
                else:
                    eng.tensor_add(o[:], G[b][a][:], G[b][b_][:])

            def TS(b, name, src, scale, bias, into=None):
                o = G[b][into] if into else T(b, name)
                if into:
                    G[b][name] = o
                e = ts_eng[name]
                if e == "a":
                    nc.scalar.activation(
                        o[:], G[b][src][:], AF.Identity,
                        bias=reg_const2(bias), scale=scale,
                    )
                else:
                    eng = nc.vector if e == "v" else nc.gpsimd
                    eng.tensor_scalar(o[:], G[b][src][:], scale, bias, MULT, ADD)

            def head(b):
                cf = b * f_blk
                for nm, dram in (("r", r_d), ("th", th_d), ("ph", ph_d)):
                    tl = io.tile(
                        [P, f_blk], f16, tag="i" + nm, bufs=io_bufs, name=f"{nm}{b}"
                    )
                    nc.sync.dma_start(tl[:], dram.ap()[:, cf : cf + f_blk])
                    G[b][nm] = tl

                # ACT tables (angular first; in-order issue)
                ACTI(b, "s", "th", AF.Sin)
                ACTI(b, "u", "th", AF.Sin, bias=HPI)
                ACTI(b, "S34", "ph", AF.Sin, bias=W["d34"])
                ACTI(b, "S4", "ph", AF.Sin, bias=W["d4"])
                ACTI(b, "S5", "ph", AF.Sin, bias=W["d5"], scale=2.0)
                ACTI(b, "S1", "ph", AF.Sin, bias=W["d1"])
                ACTI(b, "S2", "ph", AF.Sin, bias=W["d2"])
                # early DVE TS that feed the Pool J-chain
                TS(b, "um", "u", W["ro4"], -W["ro4"])
                TS(b, "J34s", "S34", W["ro34"], 0.0)
                TS(b, "Z5p", "s", W["ro5"], 0.0)
                # Pool J-chain (self-contained; in-place into um/Z5p tiles,
                # one handoff out: sJz ends up living in the um tag)
                TT(b, "uS4", "um", "S4", into="um")
                TT(b, "J", "J34s", "uS4", "a", into="um")
                TT(b, "z1", "Z5p", "S5", into="Z5p")
                TT(b, "Jz", "J", "z1", "a", into="um")
                TT(b, "sJz", "s", "Jz", into="um")
                # remaining ACT (radial exps)
                ACTI(b, "E2", "r", AF.Exp, scale=-0.5)
                ACTI(b, "E3", "r", AF.Exp, scale=float(np.float32(-1.0 / 3.0)))
                # DVE angular assembly (A accumulates in sS1, B in sS2)
                TS(b, "Au", "u", W["w11"], W["a3"])
                TT(b, "sS1", "s", "S1")
                TS(b, "K1", "sS1", W["ro1"], 0.0, into="sS1")
                TT(b, "A", "Au", "K1", "a", into="sS1")
                TS(b, "Bu", "u", W["w12"], W["a5"])
                TT(b, "sS2", "s", "S2")
                TS(b, "K2", "sS2", W["ro2"], 0.0, into="sS2")
                TT(b, "B", "Bu", "K2", "a", into="sS2")
                TS(b, "m1", "u", W["c2"], W["c1"])
                TT(b, "m2", "m1", "u", into="m1")
                TS(b, "Cc", "m2", 1.0, W["c0q"], into="m1")
                TS(b, "t", "E2", W["a1"], W["a2"])

            def tail(b):
                cf = b * f_blk
                TT(b, "C", "Cc", "sJz", "a", into="m1")
                TT(b, "rA", "r", "A", into="sS1")
                TT(b, "i2", "t", "rA", "a", into="sS1")
                TT(b, "F2", "i2", "E2", into="sS1")
                TT(b, "H1", "r", "C", into="m1")
                TT(b, "H2", "B", "H1", "a", into="m1")
                TT(b, "H3", "r", "H2", into="m1")
                TS(b, "i3", "H3", 1.0, W["a4"], into="m1")
                TT(b, "F3", "i3", "E3", into="m1")
                ot = io.tile([P, f_blk], f16, tag="ot", bufs=io_bufs, name=f"ot{b}")
                eng = nc.vector if tt_eng["out"] == "v" else nc.gpsimd
                eng.tensor_add(ot[:], G[b]["F2"][:], G[b]["F3"][:])
                # issue the store off the SP queue: an SP-issued store would
                # serialize block b+k input prefetch behind block b compute
                # (SP SEQ is in-order).  Pool issue costs only 25 ns of SEQ.
                oeng = {"a": nc.scalar, "p": nc.gpsimd, "s": nc.sync}[
                    cfg.get("odma", "p")
                ]
                oeng.dma_start(out_d.ap()[:, cf : cf + f_blk], ot[:])

            for slot in range(NB + lag):
                if slot < NB:
                    head(slot)
                if slot >= lag:
                    tail(slot - lag)

    _split_excess_waits(nc, 1)
    return nc


# --------------------------------------------------------------------------
# public entry point
# --------------------------------------------------------------------------
def kernel(position, coeffs):
    global LAST_RESULTS
    from concourse.bass_utils import run_bass_kernel_spmd

    position = np.asarray(position, dtype=np.float32)
    coeffs = np.asarray(coeffs, dtype=np.float32)
    OutN, n, _ = position.shape
    rows = OutN // N_CORES

    key = coeffs.tobytes()
    if key not in _BUILD_CACHE:
        _BUILD_CACHE[key] = _build_nc(_derive_weights(coeffs), DEFAULT_CFG)
    nc = _BUILD_CACHE[key]

    pos16 = position.astype(np.float16)
    in_maps = []
    for c in range(N_CORES):
        shard = pos16[c * rows : (c + 1) * rows].reshape(P, COLS, 3)
        in_maps.append(
            {
                "r": np.ascontiguousarray(shard[:, :, 0]),
                "th": np.ascontiguousarray(shard[:, :, 1]),
                "ph": np.ascontiguousarray(shard[:, :, 2]),
            }
        )

    res = None
    last_exc = None
    for attempt in range(3):
        try:
            res = run_bass_kernel_spmd(nc, in_maps, core_ids=list(range(N_CORES)))
            break
        except Exception as exc:  # wedged-device resilience: retry fresh
            last_exc = exc
            import time as _time

            _time.sleep(10)
    if res is None:
        raise last_exc
    LAST_RESULTS = res
    out = np.empty((OutN, n), dtype=np.float32)
    for c in range(N_CORES):
        out[c * rows : (c + 1) * rows] = (
            res.results[c]["out"].astype(np.float32).reshape(rows, n)
        )
    return out
